# revision 37
# baseline (speedup 1.0000x reference)
"""Multi-head attention (B=4, T=2048, D=1024, H=16, causal) on 8 TRN2 NeuronCores.

Sharding: tensor-parallel over heads — core c owns heads {2c, 2c+1}
(columns [128c, 128c+128) of the QKV projections, rows [128c, 128c+128) of Wo).
Each core computes q/k/v for its heads over all B*T tokens, causal attention,
and a partial output projection; the host sums the 8 partials and adds bo.

Layout: "feature-major" — activations kept as [feature, token] so every matmul
contracts over the partition dim without transposes.  Scores are computed
transposed (S_T[tk, tq]) so softmax needs no P transpose for P@V; the softmax
denominator comes free from a ones-column appended to V; normalization happens
after P@V on the small output tile.

Dtypes: x/weights/V/P/O in bf16 (f32 PSUM accumulation), qT/kT in f32r so the
score errors that exp() amplifies stay small.  bf16 halves the moving-operand
stream time on the PE (f32r streams ~1.4x slower than the 1 col/cycle rate),
allows 1024-wide moving operands in the projections, and unlocks the DVE 2x
mode for the causal-mask multiply.  Diagonal score tiles only compute/exp the
causal-visible columns.
"""
import sys

sys.path.insert(0, "/opt/trn_rl_repo")

import numpy as np
import ml_dtypes

import concourse.bacc as bacc
import concourse.tile as tile
from concourse import mybir
from concourse.bass_utils import run_bass_kernel_spmd
from concourse.masks import make_identity

B, T, D, H, HD = 4, 2048, 1024, 16, 64
NCORES = 8
DPC = 128          # dout per core = 2 heads * 64
BT = B * T         # 8192
TW = 512           # tq window width
NTG = BT // TW     # 16 token groups
NKT = D // 128     # 8 contraction tiles for projections
NWIN = T // TW     # 4 tq windows per batch
VSTRIDE = 2 * (HD + 1)  # 130: per-tk-tile V_aug columns (2 heads x (64 V + 1 ones))
SCALE = 1.0 / np.sqrt(HD)

f32 = mybir.dt.float32
f32r = mybir.dt.float32r
bf16 = mybir.dt.bfloat16
MULT = mybir.AluOpType.mult

_cache = {}


def _build(with_bias: bool, debug: bool = False):
    nc = bacc.Bacc()
    xT = nc.dram_tensor("xT", [D, BT], bf16, kind="ExternalInput")
    # qkv weights arrive pre-transposed [DPC, D] so each loads as ONE dma with
    # 2KB/partition descriptors (row-per-partition); per-kt slicing needed 24
    # small dispatches and stalled the first matmul ~20us.
    wq = nc.dram_tensor("wq", [DPC, D], bf16, kind="ExternalInput")
    wk = nc.dram_tensor("wk", [DPC, D], bf16, kind="ExternalInput")
    wv = nc.dram_tensor("wv", [DPC, D], bf16, kind="ExternalInput")
    wo = nc.dram_tensor("wo", [DPC, D], bf16, kind="ExternalInput")
    out = nc.dram_tensor("out", [D, BT], mybir.dt.float16, kind="ExternalOutput")
    if debug:
        dbg_qT = nc.dram_tensor("dbg_qT", [128, BT], f32, kind="ExternalOutput")
        dbg_kT = nc.dram_tensor("dbg_kT", [128, BT], f32, kind="ExternalOutput")
        dbg_va = nc.dram_tensor("dbg_va", [128, (BT // 128) * VSTRIDE], bf16, kind="ExternalOutput")
        dbg_oT = nc.dram_tensor("dbg_oT", [128, BT], bf16, kind="ExternalOutput")
        dbg_s = nc.dram_tensor("dbg_s", [128, 2 * TW], f32, kind="ExternalOutput")
        dbg_p = nc.dram_tensor("dbg_p", [128, 2 * TW], bf16, kind="ExternalOutput")
        dbg_ost = nc.dram_tensor("dbg_ost", [HD + 1, TW], f32, kind="ExternalOutput")
        dbg_tri = nc.dram_tensor("dbg_tri", [128, 128], bf16, kind="ExternalOutput")
    if with_bias:
        bq = nc.dram_tensor("bq", [DPC, 1], f32, kind="ExternalInput")
        bk = nc.dram_tensor("bk", [DPC, 1], f32, kind="ExternalInput")
        bv = nc.dram_tensor("bv", [DPC, 1], f32, kind="ExternalInput")

    # tri[p, f] = 1.0 if f >= p else 0.0 (keep iff tq >= tk on the diagonal block)
    tri_np = np.zeros((128, 128), dtype=np.float32)
    p_idx = np.arange(128)[:, None]
    f_idx = np.arange(128)[None, :]
    tri_np[f_idx >= p_idx] = 1.0
    tri_dram = nc.inline_tensor(
        tri_np.astype(ml_dtypes.bfloat16).view(np.uint16), name="tri"
    )

    with tile.TileContext(nc) as tc:
        with (
            tc.tile_pool(name="pers", bufs=1) as pers,
            tc.tile_pool(name="xp", bufs=4) as xp,
            tc.tile_pool(name="vs", bufs=2) as vsp,
            tc.tile_pool(name="pp", bufs=2) as ppool,
            tc.tile_pool(name="nrm", bufs=2) as nrm,
            tc.tile_pool(name="outp", bufs=4) as outp,
        ):
            wq_sb = pers.tile([128, D], bf16, tag="wq")
            wk_sb = pers.tile([128, D], bf16, tag="wk")
            wv_sb = pers.tile([128, D], bf16, tag="wv")
            wo_sb = pers.tile([128, D], bf16, tag="wo")
            qT = pers.tile([128, BT], f32r, tag="qT")
            kT = pers.tile([128, BT], f32r, tag="kT")
            oT = pers.tile([128, BT], bf16, tag="oT")
            vaug = pers.tile([128, (BT // 128) * VSTRIDE], bf16, tag="vaug")
            tri_sb = pers.tile([128, 128], bf16, tag="tri")
            ident = pers.tile([128, 128], f32, tag="ident")

            make_identity(nc, ident[:])
            nc.sync.dma_start(tri_sb[:], tri_dram[:].bitcast(bf16))
            # ones columns of V_aug (col 64 and 129 of each VSTRIDE block)
            vaug_ones = vaug[:].rearrange(
                "p (t g w) -> p t g w", t=BT // 128, g=2
            )[:, :, :, HD : HD + 1]
            nc.gpsimd.memset(vaug_ones, 1.0)
            nc.sync.dma_start(wq_sb[:], wq[:, :])
            nc.sync.dma_start(wk_sb[:], wk[:, :])
            nc.sync.dma_start(wv_sb[:], wv[:, :])
            if with_bias:
                bq_sb = pers.tile([128, 1], f32, tag="bq")
                bk_sb = pers.tile([128, 1], f32, tag="bk")
                bv_sb = pers.tile([128, 1], f32, tag="bv")
                nc.sync.dma_start(bq_sb[:], bq[:, :])
                nc.sync.dma_start(bk_sb[:], bk[:, :])
                nc.sync.dma_start(bv_sb[:], bv[:, :])

            # Single PSUM pool, 8 banks exactly:
            #   pj (proj, 1 bank x 2 bufs) + s (scores/outproj, 2 banks x 2
            #   bufs) + o0/o1 (1 bank each).
            # QKV projection is interleaved into the attention batches: batch
            # b+1's projections are emitted inside batch b's windows, so the
            # PE-dense projection fills the exp/normalize stalls and the
            # ACT-heavy exp stream overlaps projection.
            with tc.tile_pool(name="ps2", bufs=1, space="PSUM") as ps2:

                def emit_proj_tg(tg):
                    # project one 512-token group: q -> qT (ACT copy),
                    # k -> kT (DVE), v -> v_st -> V_aug via DMA transpose
                    toks = slice(tg * TW, tg * TW + TW)
                    xs = []
                    for kt in range(NKT):
                        x_t = xp.tile([128, TW], bf16, tag="x", name="x", bufs=24)
                        nc.sync.dma_start(x_t[:], xT[kt * 128 : kt * 128 + 128, toks])
                        xs.append(x_t)
                    for w, w_sb in (("q", wq_sb), ("k", wk_sb), ("v", wv_sb)):
                        w_ps = ps2.tile([128, TW], f32, tag="pj", name=f"pj{w}", bufs=2)
                        for kt in range(NKT):
                            s = slice(kt * 128, kt * 128 + 128)
                            nc.tensor.matmul(
                                w_ps[:], w_sb[:, s], xs[kt][:],
                                start=kt == 0, stop=kt == NKT - 1,
                            )
                        if w == "q":
                            if with_bias:
                                nc.scalar.add(qT[:, toks], w_ps[:], bq_sb[:])
                            else:
                                nc.scalar.copy(qT[:, toks], w_ps[:])
                        elif w == "k":
                            if with_bias:
                                nc.vector.tensor_scalar_add(kT[:, toks], w_ps[:], bk_sb[:])
                            else:
                                nc.vector.tensor_copy(kT[:, toks], w_ps[:])
                        else:
                            v_st = vsp.tile([128, TW], f32, tag="vst")
                            if with_bias:
                                nc.vector.tensor_scalar_add(v_st[:], w_ps[:], bv_sb[:])
                            else:
                                nc.vector.tensor_copy(v_st[:], w_ps[:])

                    def finish_vt(tg=tg, v_st=v_st):
                        # PE transpose into a psum tile sharing the "pj" tag
                        # (no extra bank), then scatter into V_aug.  Deferred
                        # to the next interleave slot so the PE never waits on
                        # the v_st cast.
                        vt_ps = ps2.tile([128, TW], f32, tag="pj", name="vt", bufs=2)
                        for j4 in range(TW // 128):
                            nc.tensor.transpose(
                                vt_ps[:, j4 * 128 : j4 * 128 + 128],
                                v_st[:, j4 * 128 : j4 * 128 + 128],
                                ident[:],
                            )
                        c = tg * 4 * VSTRIDE
                        dst = vaug[:, c : c + 4 * VSTRIDE].rearrange(
                            "p (t g w) -> p t g w", t=4, g=2
                        )[:, :, :, 0:HD]
                        srcv = vt_ps[:].rearrange("p (t g w) -> p t g w", t=4, g=2)
                        nc.vector.tensor_copy(dst, srcv)

                    return finish_vt

                def emit_outproj_tg(tg):
                    # partial output projection for one 512-token group; emitted
                    # one batch late, interleaved into attention windows so its
                    # PE/copy work fills the exp/normalize dependency stalls.
                    tok = slice(tg * TW, tg * TW + TW)
                    for dmg in range(4):
                        pr = ps2.tile([128, 2 * TW], f32, tag="s", name="pr", bufs=2)
                        for dmi in range(2):
                            dm = dmg * 2 + dmi
                            s = slice(dm * 128, dm * 128 + 128)
                            nc.tensor.matmul(
                                pr[:, dmi * TW : dmi * TW + TW],
                                wo_sb[:, s],
                                oT[:, tok],
                                start=True,
                                stop=True,
                            )
                        st2 = outp.tile([128, 2 * TW], mybir.dt.float16, tag="st")
                        if dmg % 4 == 3:
                            nc.scalar.copy(st2[:], pr[:])
                        else:
                            nc.vector.tensor_copy(st2[:], pr[:])
                        for dmi in range(2):
                            dm = dmg * 2 + dmi
                            s = slice(dm * 128, dm * 128 + 128)
                            nc.sync.dma_start(
                                out[s, tok], st2[:, dmi * TW : dmi * TW + TW]
                            )

                pending_vt = None
                for tg in range(4):
                    f = emit_proj_tg(tg)
                    if pending_vt is not None:
                        pending_vt()
                    pending_vt = f
                nc.sync.dma_start(wo_sb[:], wo[:, :])

                for b in range(B):
                    tb = b * T
                    for wi in range(NWIN):
                        win = slice(tb + wi * TW, tb + wi * TW + TW)
                        jmax = 4 * wi + 4
                        o_ps = [
                            ps2.tile([HD + 1, TW], f32, tag=f"o{h}", name=f"o{h}")
                            for h in (0, 1)
                        ]
                        # j's processed in software-pipelined pairs: scores of
                        # pair c (64-row mode back-to-back), exps of c, then PV
                        # accumulations of pair c-1 (128-row mode) — halves the
                        # PE row-tiling mode switches and keeps the PE off the
                        # exp critical path.
                        def emit_pvs(plist):
                            for j, c0, p_pr in plist:
                                vcol = ((tb // 128) + j) * VSTRIDE
                                for h in (0, 1):
                                    nc.tensor.matmul(
                                        o_ps[h][:, c0:TW],
                                        vaug[:, vcol + h * (HD + 1) : vcol + h * (HD + 1) + HD + 1],
                                        p_pr[:, h * TW + c0 : h * TW + TW],
                                        start=(j == 0),
                                        stop=(j == jmax - 1),
                                    )

                        prev = []
                        for jc in range(0, jmax, 2):
                            cur = []
                            for j in range(jc, min(jc + 2, jmax)):
                                bj = slice(tb + j * 128, tb + j * 128 + 128)
                                d = j - 4 * wi
                                c0 = 128 * d if d > 0 else 0
                                s_pr = ps2.tile([128, 2 * TW], f32, tag="s", bufs=2)
                                for h in (0, 1):
                                    nc.tensor.matmul(
                                        s_pr[:, h * TW + c0 : h * TW + TW],
                                        kT[h * HD : h * HD + HD, bj],
                                        qT[h * HD : h * HD + HD, win.start + c0 : win.stop],
                                        start=True,
                                        stop=True,
                                    )
                                cur.append((j, d, c0, s_pr))
                            pcur = []
                            for j, d, c0, s_pr in cur:
                                p_pr = ppool.tile([128, 2 * TW], bf16, tag="p", bufs=6)
                                if c0 == 0:
                                    nc.scalar.activation(
                                        p_pr[:],
                                        s_pr[:],
                                        mybir.ActivationFunctionType.Exp,
                                        scale=float(SCALE),
                                    )
                                else:
                                    p_v = p_pr[:].rearrange("p (g w) -> p g w", g=2)[:, :, c0:TW]
                                    s_v = s_pr[:].rearrange("p (g w) -> p g w", g=2)[:, :, c0:TW]
                                    nc.scalar.activation(
                                        p_v,
                                        s_v,
                                        mybir.ActivationFunctionType.Exp,
                                        scale=float(SCALE),
                                    )
                                if d >= 0:  # diagonal: zero strict lower triangle
                                    for h in (0, 1):
                                        ts = slice(
                                            h * TW + 128 * d, h * TW + 128 * d + 128
                                        )
                                        nc.vector.tensor_tensor(
                                            p_pr[:, ts], p_pr[:, ts], tri_sb[:], MULT
                                        )
                                pcur.append((j, c0, p_pr))
                            emit_pvs(prev)
                            prev = pcur
                        emit_pvs(prev)
                        for h in (0, 1):
                            o_st = nrm.tile([HD + 1, TW], f32, tag="ost", bufs=6)
                            nc.vector.tensor_copy(o_st[:], o_ps[h][:])
                            den0 = nrm.tile([1, TW], f32, tag="den0", bufs=3)
                            nc.sync.dma_start(den0[:], o_st[HD : HD + 1, :])
                            bc = nrm.tile([HD, TW], f32, tag="bc", bufs=3)
                            nc.gpsimd.partition_broadcast(bc[:], den0[0:1, :])
                            if debug and b == 0 and wi == 0 and h == 0:
                                nc.sync.dma_start(dbg_ost[:, :], o_st[:])
                            rc = nrm.tile([HD, TW], f32, tag="rc", bufs=3)
                            nc.vector.reciprocal_approx_fast(out=rc[:], in_=bc[:])
                            nc.vector.tensor_tensor(
                                oT[h * HD : h * HD + HD, win], o_st[0:HD, :], rc[:], MULT
                            )
                        if b > 0:
                            emit_outproj_tg(4 * (b - 1) + wi)
                        if b == B - 1 and wi > 0:
                            # last batch: previous window's oT is normalized by
                            # now — emit its outproj here instead of the tail
                            emit_outproj_tg(4 * (B - 1) + wi - 1)
                        if pending_vt is not None:
                            pending_vt()
                            pending_vt = None
                        if b < B - 1:
                            pending_vt = emit_proj_tg(4 * (b + 1) + wi)

                emit_outproj_tg(4 * B - 1)

            if debug:
                nc.sync.dma_start(dbg_qT[:, :], qT[:].bitcast(f32))
                nc.sync.dma_start(dbg_kT[:, :], kT[:].bitcast(f32))
                nc.sync.dma_start(dbg_va[:, :], vaug[:])
                nc.sync.dma_start(dbg_oT[:, :], oT[:])
                nc.sync.dma_start(dbg_tri[:, :], tri_sb[:])

    nc.compile()
    return nc


def _get_nc(with_bias: bool, debug: bool = False):
    key = (with_bias, debug)
    if key not in _cache:
        _cache[key] = _build(with_bias, debug)
    return _cache[key]


def _make_in_maps(x, Wq, bq, Wk, bk, Wv, bv, Wo, with_bias):
    bf = ml_dtypes.bfloat16
    xT = np.ascontiguousarray(x.reshape(BT, D).T.astype(bf))
    in_maps = []
    for c in range(NCORES):
        cs = slice(c * DPC, c * DPC + DPC)
        def _blockT(w):
            # [D, DPC] -> [128, D]: partition p holds, per kt block, row
            # (kt*128 + p) of w — the layout wq_sb[:, kt*128:+128] expects
            # (partitions = contraction slice, cols = output features).
            return np.ascontiguousarray(
                w.astype(bf).reshape(NKT, 128, DPC).transpose(1, 0, 2).reshape(128, D)
            )

        m = {
            "xT": xT,
            "wq": _blockT(Wq[:, cs]),
            "wk": _blockT(Wk[:, cs]),
            "wv": _blockT(Wv[:, cs]),
            "wo": np.ascontiguousarray(Wo[cs, :].astype(bf)),
        }
        if with_bias:
            m["bq"] = np.ascontiguousarray(bq[cs]).reshape(DPC, 1)
            m["bk"] = np.ascontiguousarray(bk[cs]).reshape(DPC, 1)
            m["bv"] = np.ascontiguousarray(bv[cs]).reshape(DPC, 1)
        in_maps.append(m)
    return in_maps


def _gather(res, bo):
    acc = np.zeros((D, BT), dtype=np.float32)
    for r in res.results:
        acc += r["out"].astype(np.float32)
    y = acc.T + bo[None, :]
    return np.ascontiguousarray(y.reshape(B, T, D), dtype=np.float32)


def kernel(x, Wq, bq, Wk, bk, Wv, bv, Wo, bo, _trace=False):
    x = np.asarray(x, dtype=np.float32)
    Wq, Wk, Wv, Wo = (np.asarray(w, dtype=np.float32) for w in (Wq, Wk, Wv, Wo))
    bq, bk, bv, bo = (np.asarray(b_, dtype=np.float32) for b_ in (bq, bk, bv, bo))

    with_bias = bool(np.any(bq != 0) or np.any(bk != 0) or np.any(bv != 0))
    nc = _get_nc(with_bias)
    in_maps = _make_in_maps(x, Wq, bq, Wk, bk, Wv, bv, Wo, with_bias)
    res = run_bass_kernel_spmd(
        nc, in_maps, core_ids=list(range(NCORES)), trace=_trace
    )
    y = _gather(res, bo)
    if _trace:
        return y, res
    return y


# revision 38
# speedup vs baseline: 1.0207x; 1.0207x over previous
"""Multi-head attention (B=4, T=2048, D=1024, H=16, causal) on 8 TRN2 NeuronCores.

Sharding: tensor-parallel over heads — core c owns heads {2c, 2c+1}
(columns [128c, 128c+128) of the QKV projections, rows [128c, 128c+128) of Wo).
Each core computes q/k/v for its heads over all B*T tokens, causal attention,
and a partial output projection; the host sums the 8 partials and adds bo.

Layout: "feature-major" — activations kept as [feature, token] so every matmul
contracts over the partition dim without transposes.  Scores are computed
transposed (S_T[tk, tq]) so softmax needs no P transpose for P@V; the softmax
denominator comes free from a ones-column appended to V; normalization happens
after P@V on the small output tile.

Dtypes: x/weights/V/P/O in bf16 (f32 PSUM accumulation), qT/kT in f32r so the
score errors that exp() amplifies stay small.  bf16 halves the moving-operand
stream time on the PE (f32r streams ~1.4x slower than the 1 col/cycle rate),
allows 1024-wide moving operands in the projections, and unlocks the DVE 2x
mode for the causal-mask multiply.  Diagonal score tiles only compute/exp the
causal-visible columns.
"""
import sys

sys.path.insert(0, "/opt/trn_rl_repo")

import numpy as np
import ml_dtypes

import concourse.bacc as bacc
import concourse.tile as tile
from concourse import mybir
from concourse.bass_utils import run_bass_kernel_spmd
from concourse.masks import make_identity

B, T, D, H, HD = 4, 2048, 1024, 16, 64
NCORES = 8
DPC = 128          # dout per core = 2 heads * 64
BT = B * T         # 8192
TW = 512           # tq window width
NTG = BT // TW     # 16 token groups
NKT = D // 128     # 8 contraction tiles for projections
NWIN = T // TW     # 4 tq windows per batch
VSTRIDE = 2 * (HD + 1)  # 130: per-tk-tile V_aug columns (2 heads x (64 V + 1 ones))
SCALE = 1.0 / np.sqrt(HD)

f32 = mybir.dt.float32
f32r = mybir.dt.float32r
bf16 = mybir.dt.bfloat16
MULT = mybir.AluOpType.mult

_cache = {}


def _build(with_bias: bool, debug: bool = False):
    nc = bacc.Bacc()
    xT = nc.dram_tensor("xT", [D, BT], bf16, kind="ExternalInput")
    # qkv weights arrive pre-transposed [DPC, D] so each loads as ONE dma with
    # 2KB/partition descriptors (row-per-partition); per-kt slicing needed 24
    # small dispatches and stalled the first matmul ~20us.
    wq = nc.dram_tensor("wq", [DPC, D], bf16, kind="ExternalInput")
    wk = nc.dram_tensor("wk", [DPC, D], bf16, kind="ExternalInput")
    wv = nc.dram_tensor("wv", [DPC, D], bf16, kind="ExternalInput")
    wo = nc.dram_tensor("wo", [DPC, D], bf16, kind="ExternalInput")
    out = nc.dram_tensor("out", [D, BT], mybir.dt.float16, kind="ExternalOutput")
    if debug:
        dbg_qT = nc.dram_tensor("dbg_qT", [128, BT], f32, kind="ExternalOutput")
        dbg_kT = nc.dram_tensor("dbg_kT", [128, BT], f32, kind="ExternalOutput")
        dbg_va = nc.dram_tensor("dbg_va", [128, (BT // 128) * VSTRIDE], bf16, kind="ExternalOutput")
        dbg_oT = nc.dram_tensor("dbg_oT", [128, BT], bf16, kind="ExternalOutput")
        dbg_s = nc.dram_tensor("dbg_s", [128, 2 * TW], f32, kind="ExternalOutput")
        dbg_p = nc.dram_tensor("dbg_p", [128, 2 * TW], bf16, kind="ExternalOutput")
        dbg_ost = nc.dram_tensor("dbg_ost", [HD + 1, TW], f32, kind="ExternalOutput")
        dbg_tri = nc.dram_tensor("dbg_tri", [128, 128], bf16, kind="ExternalOutput")
    if with_bias:
        bq = nc.dram_tensor("bq", [DPC, 1], f32, kind="ExternalInput")
        bk = nc.dram_tensor("bk", [DPC, 1], f32, kind="ExternalInput")
        bv = nc.dram_tensor("bv", [DPC, 1], f32, kind="ExternalInput")

    # tri[p, f] = 1.0 if f >= p else 0.0 (keep iff tq >= tk on the diagonal block)
    tri_np = np.zeros((128, 128), dtype=np.float32)
    p_idx = np.arange(128)[:, None]
    f_idx = np.arange(128)[None, :]
    tri_np[f_idx >= p_idx] = 1.0
    tri_dram = nc.inline_tensor(
        tri_np.astype(ml_dtypes.bfloat16).view(np.uint16), name="tri"
    )

    with tile.TileContext(nc) as tc:
        with (
            tc.tile_pool(name="pers", bufs=1) as pers,
            tc.tile_pool(name="xp", bufs=4) as xp,
            tc.tile_pool(name="vs", bufs=2) as vsp,
            tc.tile_pool(name="pp", bufs=2) as ppool,
            tc.tile_pool(name="nrm", bufs=2) as nrm,
            tc.tile_pool(name="outp", bufs=4) as outp,
        ):
            wq_sb = pers.tile([128, D], bf16, tag="wq")
            wk_sb = pers.tile([128, D], bf16, tag="wk")
            wv_sb = pers.tile([128, D], bf16, tag="wv")
            wo_sb = pers.tile([128, D], bf16, tag="wo")
            qT = pers.tile([128, BT], f32r, tag="qT")
            kT = pers.tile([128, BT], f32r, tag="kT")
            oT = pers.tile([128, BT], bf16, tag="oT")
            vaug = pers.tile([128, (BT // 128) * VSTRIDE], bf16, tag="vaug")
            tri_sb = pers.tile([128, 128], bf16, tag="tri")
            ident = pers.tile([128, 128], f32, tag="ident")

            make_identity(nc, ident[:])
            nc.sync.dma_start(tri_sb[:], tri_dram[:].bitcast(bf16))
            # ones columns of V_aug (col 64 and 129 of each VSTRIDE block)
            vaug_ones = vaug[:].rearrange(
                "p (t g w) -> p t g w", t=BT // 128, g=2
            )[:, :, :, HD : HD + 1]
            nc.gpsimd.memset(vaug_ones, 1.0)
            nc.sync.dma_start(wq_sb[:], wq[:, :])
            nc.sync.dma_start(wk_sb[:], wk[:, :])
            nc.sync.dma_start(wv_sb[:], wv[:, :])
            if with_bias:
                bq_sb = pers.tile([128, 1], f32, tag="bq")
                bk_sb = pers.tile([128, 1], f32, tag="bk")
                bv_sb = pers.tile([128, 1], f32, tag="bv")
                nc.sync.dma_start(bq_sb[:], bq[:, :])
                nc.sync.dma_start(bk_sb[:], bk[:, :])
                nc.sync.dma_start(bv_sb[:], bv[:, :])

            # Single PSUM pool, 8 banks exactly:
            #   pj (proj, 1 bank x 2 bufs) + s (scores/outproj, 2 banks x 2
            #   bufs) + o0/o1 (1 bank each).
            # QKV projection is interleaved into the attention batches: batch
            # b+1's projections are emitted inside batch b's windows, so the
            # PE-dense projection fills the exp/normalize stalls and the
            # ACT-heavy exp stream overlaps projection.
            with tc.tile_pool(name="ps2", bufs=1, space="PSUM") as ps2:

                def emit_proj_tg(tg):
                    # project one 512-token group: q -> qT (ACT copy),
                    # k -> kT (DVE), v -> v_st -> V_aug via DMA transpose
                    toks = slice(tg * TW, tg * TW + TW)
                    xs = []
                    for kt in range(NKT):
                        x_t = xp.tile([128, TW], bf16, tag="x", name="x", bufs=24)
                        nc.sync.dma_start(x_t[:], xT[kt * 128 : kt * 128 + 128, toks])
                        xs.append(x_t)
                    for w, w_sb in (("q", wq_sb), ("k", wk_sb), ("v", wv_sb)):
                        w_ps = ps2.tile([128, TW], f32, tag="pj", name=f"pj{w}", bufs=2)
                        for kt in range(NKT):
                            s = slice(kt * 128, kt * 128 + 128)
                            nc.tensor.matmul(
                                w_ps[:], w_sb[:, s], xs[kt][:],
                                start=kt == 0, stop=kt == NKT - 1,
                            )
                        if w == "q":
                            if with_bias:
                                nc.scalar.add(qT[:, toks], w_ps[:], bq_sb[:])
                            else:
                                nc.scalar.copy(qT[:, toks], w_ps[:])
                        elif w == "k":
                            if with_bias:
                                nc.vector.tensor_scalar_add(kT[:, toks], w_ps[:], bk_sb[:])
                            else:
                                nc.vector.tensor_copy(kT[:, toks], w_ps[:])
                        else:
                            v_st = vsp.tile([128, TW], f32, tag="vst")
                            if with_bias:
                                nc.vector.tensor_scalar_add(v_st[:], w_ps[:], bv_sb[:])
                            else:
                                nc.vector.tensor_copy(v_st[:], w_ps[:])
                            # PE transpose into a psum tile sharing the "pj"
                            # tag (no extra bank), then scatter into V_aug
                            vt_ps = ps2.tile([128, TW], f32, tag="pj", name="vt", bufs=2)
                            for j4 in range(TW // 128):
                                nc.tensor.transpose(
                                    vt_ps[:, j4 * 128 : j4 * 128 + 128],
                                    v_st[:, j4 * 128 : j4 * 128 + 128],
                                    ident[:],
                                )
                            c = tg * 4 * VSTRIDE
                            dst = vaug[:, c : c + 4 * VSTRIDE].rearrange(
                                "p (t g w) -> p t g w", t=4, g=2
                            )[:, :, :, 0:HD]
                            srcv = vt_ps[:].rearrange("p (t g w) -> p t g w", t=4, g=2)
                            nc.vector.tensor_copy(dst, srcv)

                def emit_outproj_tg(tg):
                    # partial output projection for one 512-token group; emitted
                    # one batch late, interleaved into attention windows so its
                    # PE/copy work fills the exp/normalize dependency stalls.
                    tok = slice(tg * TW, tg * TW + TW)
                    for dmg in range(4):
                        pr = ps2.tile([128, 2 * TW], f32, tag="s", name="pr", bufs=2)
                        for dmi in range(2):
                            dm = dmg * 2 + dmi
                            s = slice(dm * 128, dm * 128 + 128)
                            nc.tensor.matmul(
                                pr[:, dmi * TW : dmi * TW + TW],
                                wo_sb[:, s],
                                oT[:, tok],
                                start=True,
                                stop=True,
                            )
                        st2 = outp.tile([128, 2 * TW], mybir.dt.float16, tag="st")
                        if dmg % 4 == 3:
                            nc.scalar.copy(st2[:], pr[:])
                        else:
                            nc.vector.tensor_copy(st2[:], pr[:])
                        for dmi in range(2):
                            dm = dmg * 2 + dmi
                            s = slice(dm * 128, dm * 128 + 128)
                            nc.sync.dma_start(
                                out[s, tok], st2[:, dmi * TW : dmi * TW + TW]
                            )

                for tg in range(4):
                    emit_proj_tg(tg)
                nc.sync.dma_start(wo_sb[:], wo[:, :])

                for b in range(B):
                    tb = b * T
                    for wi in range(NWIN):
                        win = slice(tb + wi * TW, tb + wi * TW + TW)
                        jmax = 4 * wi + 4
                        o_ps = [
                            ps2.tile([HD + 1, TW], f32, tag=f"o{h}", name=f"o{h}")
                            for h in (0, 1)
                        ]
                        # j's processed in software-pipelined pairs: scores of
                        # pair c (64-row mode back-to-back), exps of c, then PV
                        # accumulations of pair c-1 (128-row mode) — halves the
                        # PE row-tiling mode switches and keeps the PE off the
                        # exp critical path.
                        def emit_pvs(plist):
                            for j, c0, p_pr in plist:
                                vcol = ((tb // 128) + j) * VSTRIDE
                                for h in (0, 1):
                                    nc.tensor.matmul(
                                        o_ps[h][:, c0:TW],
                                        vaug[:, vcol + h * (HD + 1) : vcol + h * (HD + 1) + HD + 1],
                                        p_pr[:, h * TW + c0 : h * TW + TW],
                                        start=(j == 0),
                                        stop=(j == jmax - 1),
                                    )

                        prev = []
                        for jc in range(0, jmax, 2):
                            cur = []
                            for j in range(jc, min(jc + 2, jmax)):
                                bj = slice(tb + j * 128, tb + j * 128 + 128)
                                d = j - 4 * wi
                                c0 = 128 * d if d > 0 else 0
                                s_pr = ps2.tile([128, 2 * TW], f32, tag="s", bufs=2)
                                for h in (0, 1):
                                    nc.tensor.matmul(
                                        s_pr[:, h * TW + c0 : h * TW + TW],
                                        kT[h * HD : h * HD + HD, bj],
                                        qT[h * HD : h * HD + HD, win.start + c0 : win.stop],
                                        start=True,
                                        stop=True,
                                    )
                                cur.append((j, d, c0, s_pr))
                            pcur = []
                            for j, d, c0, s_pr in cur:
                                p_pr = ppool.tile([128, 2 * TW], bf16, tag="p", bufs=6)
                                if c0 == 0:
                                    nc.scalar.activation(
                                        p_pr[:],
                                        s_pr[:],
                                        mybir.ActivationFunctionType.Exp,
                                        scale=float(SCALE),
                                    )
                                else:
                                    p_v = p_pr[:].rearrange("p (g w) -> p g w", g=2)[:, :, c0:TW]
                                    s_v = s_pr[:].rearrange("p (g w) -> p g w", g=2)[:, :, c0:TW]
                                    nc.scalar.activation(
                                        p_v,
                                        s_v,
                                        mybir.ActivationFunctionType.Exp,
                                        scale=float(SCALE),
                                    )
                                if d >= 0:  # diagonal: zero strict lower triangle
                                    for h in (0, 1):
                                        ts = slice(
                                            h * TW + 128 * d, h * TW + 128 * d + 128
                                        )
                                        nc.vector.tensor_tensor(
                                            p_pr[:, ts], p_pr[:, ts], tri_sb[:], MULT
                                        )
                                pcur.append((j, c0, p_pr))
                            emit_pvs(prev)
                            prev = pcur
                        emit_pvs(prev)
                        for h in (0, 1):
                            o_st = nrm.tile([HD + 1, TW], f32, tag="ost", bufs=6)
                            nc.vector.tensor_copy(o_st[:], o_ps[h][:])
                            den0 = nrm.tile([1, TW], f32, tag="den0", bufs=3)
                            nc.sync.dma_start(den0[:], o_st[HD : HD + 1, :])
                            bc = nrm.tile([HD, TW], f32, tag="bc", bufs=3)
                            nc.gpsimd.partition_broadcast(bc[:], den0[0:1, :])
                            if debug and b == 0 and wi == 0 and h == 0:
                                nc.sync.dma_start(dbg_ost[:, :], o_st[:])
                            rc = nrm.tile([HD, TW], f32, tag="rc", bufs=3)
                            nc.vector.reciprocal_approx_fast(out=rc[:], in_=bc[:])
                            nc.vector.tensor_tensor(
                                oT[h * HD : h * HD + HD, win], o_st[0:HD, :], rc[:], MULT
                            )
                        if b > 0:
                            emit_outproj_tg(4 * (b - 1) + wi)
                        if b < B - 1:
                            emit_proj_tg(4 * (b + 1) + wi)

                for wi in range(NWIN):
                    emit_outproj_tg(4 * (B - 1) + wi)

            if debug:
                nc.sync.dma_start(dbg_qT[:, :], qT[:].bitcast(f32))
                nc.sync.dma_start(dbg_kT[:, :], kT[:].bitcast(f32))
                nc.sync.dma_start(dbg_va[:, :], vaug[:])
                nc.sync.dma_start(dbg_oT[:, :], oT[:])
                nc.sync.dma_start(dbg_tri[:, :], tri_sb[:])

    nc.compile()
    return nc


def _get_nc(with_bias: bool, debug: bool = False):
    key = (with_bias, debug)
    if key not in _cache:
        _cache[key] = _build(with_bias, debug)
    return _cache[key]


def _make_in_maps(x, Wq, bq, Wk, bk, Wv, bv, Wo, with_bias):
    bf = ml_dtypes.bfloat16
    xT = np.ascontiguousarray(x.reshape(BT, D).T.astype(bf))
    in_maps = []
    for c in range(NCORES):
        cs = slice(c * DPC, c * DPC + DPC)
        def _blockT(w):
            # [D, DPC] -> [128, D]: partition p holds, per kt block, row
            # (kt*128 + p) of w — the layout wq_sb[:, kt*128:+128] expects
            # (partitions = contraction slice, cols = output features).
            return np.ascontiguousarray(
                w.astype(bf).reshape(NKT, 128, DPC).transpose(1, 0, 2).reshape(128, D)
            )

        m = {
            "xT": xT,
            "wq": _blockT(Wq[:, cs]),
            "wk": _blockT(Wk[:, cs]),
            "wv": _blockT(Wv[:, cs]),
            "wo": np.ascontiguousarray(Wo[cs, :].astype(bf)),
        }
        if with_bias:
            m["bq"] = np.ascontiguousarray(bq[cs]).reshape(DPC, 1)
            m["bk"] = np.ascontiguousarray(bk[cs]).reshape(DPC, 1)
            m["bv"] = np.ascontiguousarray(bv[cs]).reshape(DPC, 1)
        in_maps.append(m)
    return in_maps


def _gather(res, bo):
    acc = np.zeros((D, BT), dtype=np.float32)
    for r in res.results:
        acc += r["out"].astype(np.float32)
    y = acc.T + bo[None, :]
    return np.ascontiguousarray(y.reshape(B, T, D), dtype=np.float32)


def kernel(x, Wq, bq, Wk, bk, Wv, bv, Wo, bo, _trace=False):
    x = np.asarray(x, dtype=np.float32)
    Wq, Wk, Wv, Wo = (np.asarray(w, dtype=np.float32) for w in (Wq, Wk, Wv, Wo))
    bq, bk, bv, bo = (np.asarray(b_, dtype=np.float32) for b_ in (bq, bk, bv, bo))

    with_bias = bool(np.any(bq != 0) or np.any(bk != 0) or np.any(bv != 0))
    nc = _get_nc(with_bias)
    in_maps = _make_in_maps(x, Wq, bq, Wk, bk, Wv, bv, Wo, with_bias)
    res = run_bass_kernel_spmd(
        nc, in_maps, core_ids=list(range(NCORES)), trace=_trace
    )
    y = _gather(res, bo)
    if _trace:
        return y, res
    return y


# revision 39
# speedup vs baseline: 1.0658x; 1.0442x over previous
"""Multi-head attention (B=4, T=2048, D=1024, H=16, causal) on 8 TRN2 NeuronCores.

Sharding: tensor-parallel over heads — core c owns heads {2c, 2c+1}
(columns [128c, 128c+128) of the QKV projections, rows [128c, 128c+128) of Wo).
Each core computes q/k/v for its heads over all B*T tokens, causal attention,
and a partial output projection; the host sums the 8 partials and adds bo.

Layout: "feature-major" — activations kept as [feature, token] so every matmul
contracts over the partition dim without transposes.  Scores are computed
transposed (S_T[tk, tq]) so softmax needs no P transpose for P@V; the softmax
denominator comes free from a ones-column appended to V; normalization happens
after P@V on the small output tile.

Dtypes: x/weights/V/P/O in bf16 (f32 PSUM accumulation), qT/kT in f32r so the
score errors that exp() amplifies stay small.  bf16 halves the moving-operand
stream time on the PE (f32r streams ~1.4x slower than the 1 col/cycle rate),
allows 1024-wide moving operands in the projections, and unlocks the DVE 2x
mode for the causal-mask multiply.  Diagonal score tiles only compute/exp the
causal-visible columns.
"""
import sys

sys.path.insert(0, "/opt/trn_rl_repo")

import numpy as np
import ml_dtypes

import concourse.bacc as bacc
import concourse.tile as tile
from concourse import mybir
from concourse.bass_utils import run_bass_kernel_spmd
from concourse.masks import make_identity

B, T, D, H, HD = 4, 2048, 1024, 16, 64
NCORES = 8
DPC = 128          # dout per core = 2 heads * 64
BT = B * T         # 8192
TW = 512           # tq window width
NTG = BT // TW     # 16 token groups
NKT = D // 128     # 8 contraction tiles for projections
NWIN = T // TW     # 4 tq windows per batch
VSTRIDE = 2 * (HD + 1)  # 130: per-tk-tile V_aug columns (2 heads x (64 V + 1 ones))
SCALE = 1.0 / np.sqrt(HD)

f32 = mybir.dt.float32
f32r = mybir.dt.float32r
bf16 = mybir.dt.bfloat16
MULT = mybir.AluOpType.mult

_cache = {}


def _build(with_bias: bool, debug: bool = False):
    nc = bacc.Bacc()
    xT = nc.dram_tensor("xT", [D, BT], bf16, kind="ExternalInput")
    # qkv weights arrive pre-transposed [DPC, D] so each loads as ONE dma with
    # 2KB/partition descriptors (row-per-partition); per-kt slicing needed 24
    # small dispatches and stalled the first matmul ~20us.
    wq = nc.dram_tensor("wq", [DPC, D], bf16, kind="ExternalInput")
    wk = nc.dram_tensor("wk", [DPC, D], bf16, kind="ExternalInput")
    wv = nc.dram_tensor("wv", [DPC, D], bf16, kind="ExternalInput")
    wo = nc.dram_tensor("wo", [DPC, D], bf16, kind="ExternalInput")
    out = nc.dram_tensor("out", [D, BT], mybir.dt.float16, kind="ExternalOutput")
    if debug:
        dbg_qT = nc.dram_tensor("dbg_qT", [128, BT], f32, kind="ExternalOutput")
        dbg_kT = nc.dram_tensor("dbg_kT", [128, BT], f32, kind="ExternalOutput")
        dbg_va = nc.dram_tensor("dbg_va", [128, (BT // 128) * VSTRIDE], bf16, kind="ExternalOutput")
        dbg_oT = nc.dram_tensor("dbg_oT", [128, BT], bf16, kind="ExternalOutput")
        dbg_s = nc.dram_tensor("dbg_s", [128, 2 * TW], f32, kind="ExternalOutput")
        dbg_p = nc.dram_tensor("dbg_p", [128, 2 * TW], bf16, kind="ExternalOutput")
        dbg_ost = nc.dram_tensor("dbg_ost", [HD + 1, TW], f32, kind="ExternalOutput")
        dbg_tri = nc.dram_tensor("dbg_tri", [128, 128], bf16, kind="ExternalOutput")
    if with_bias:
        bq = nc.dram_tensor("bq", [DPC, 1], f32, kind="ExternalInput")
        bk = nc.dram_tensor("bk", [DPC, 1], f32, kind="ExternalInput")
        bv = nc.dram_tensor("bv", [DPC, 1], f32, kind="ExternalInput")

    # tri[p, f] = 1.0 if f >= p else 0.0 (keep iff tq >= tk on the diagonal block)
    tri_np = np.zeros((128, 128), dtype=np.float32)
    p_idx = np.arange(128)[:, None]
    f_idx = np.arange(128)[None, :]
    tri_np[f_idx >= p_idx] = 1.0
    tri_dram = nc.inline_tensor(
        tri_np.astype(ml_dtypes.bfloat16).view(np.uint16), name="tri"
    )

    with tile.TileContext(nc) as tc:
        with (
            tc.tile_pool(name="pers", bufs=1) as pers,
            tc.tile_pool(name="xp", bufs=4) as xp,
            tc.tile_pool(name="vs", bufs=2) as vsp,
            tc.tile_pool(name="pp", bufs=2) as ppool,
            tc.tile_pool(name="nrm", bufs=2) as nrm,
            tc.tile_pool(name="outp", bufs=4) as outp,
        ):
            wq_sb = pers.tile([128, D], bf16, tag="wq")
            wk_sb = pers.tile([128, D], bf16, tag="wk")
            wv_sb = pers.tile([128, D], bf16, tag="wv")
            wo_sb = pers.tile([128, D], bf16, tag="wo")
            qT = pers.tile([128, BT], bf16, tag="qT")
            kT = pers.tile([128, BT], bf16, tag="kT")
            oT = pers.tile([128, BT], bf16, tag="oT")
            vaug = pers.tile([128, (BT // 128) * VSTRIDE], bf16, tag="vaug")
            tri_sb = pers.tile([128, 128], bf16, tag="tri")
            ident = pers.tile([128, 128], f32, tag="ident")

            make_identity(nc, ident[:])
            nc.sync.dma_start(tri_sb[:], tri_dram[:].bitcast(bf16))
            # ones columns of V_aug (col 64 and 129 of each VSTRIDE block)
            vaug_ones = vaug[:].rearrange(
                "p (t g w) -> p t g w", t=BT // 128, g=2
            )[:, :, :, HD : HD + 1]
            nc.gpsimd.memset(vaug_ones, 1.0)
            nc.sync.dma_start(wq_sb[:], wq[:, :])
            nc.sync.dma_start(wk_sb[:], wk[:, :])
            nc.sync.dma_start(wv_sb[:], wv[:, :])
            if with_bias:
                bq_sb = pers.tile([128, 1], f32, tag="bq")
                bk_sb = pers.tile([128, 1], f32, tag="bk")
                bv_sb = pers.tile([128, 1], f32, tag="bv")
                nc.sync.dma_start(bq_sb[:], bq[:, :])
                nc.sync.dma_start(bk_sb[:], bk[:, :])
                nc.sync.dma_start(bv_sb[:], bv[:, :])

            # Single PSUM pool, 8 banks exactly:
            #   pj (proj, 1 bank x 2 bufs) + s (scores/outproj, 2 banks x 2
            #   bufs) + o0/o1 (1 bank each).
            # QKV projection is interleaved into the attention batches: batch
            # b+1's projections are emitted inside batch b's windows, so the
            # PE-dense projection fills the exp/normalize stalls and the
            # ACT-heavy exp stream overlaps projection.
            with tc.tile_pool(name="ps2", bufs=1, space="PSUM") as ps2:

                def emit_proj_tg(tg):
                    # project one 512-token group: q -> qT (ACT copy),
                    # k -> kT (DVE), v -> v_st -> V_aug via DMA transpose
                    toks = slice(tg * TW, tg * TW + TW)
                    xs = []
                    for kt in range(NKT):
                        x_t = xp.tile([128, TW], bf16, tag="x", name="x", bufs=24)
                        nc.sync.dma_start(x_t[:], xT[kt * 128 : kt * 128 + 128, toks])
                        xs.append(x_t)
                    for w, w_sb in (("q", wq_sb), ("k", wk_sb), ("v", wv_sb)):
                        w_ps = ps2.tile([128, TW], f32, tag="pj", name=f"pj{w}", bufs=2)
                        for kt in range(NKT):
                            s = slice(kt * 128, kt * 128 + 128)
                            nc.tensor.matmul(
                                w_ps[:], w_sb[:, s], xs[kt][:],
                                start=kt == 0, stop=kt == NKT - 1,
                            )
                        if w == "q":
                            if with_bias:
                                nc.scalar.add(qT[:, toks], w_ps[:], bq_sb[:])
                            else:
                                nc.scalar.copy(qT[:, toks], w_ps[:])
                        elif w == "k":
                            if with_bias:
                                nc.vector.tensor_scalar_add(kT[:, toks], w_ps[:], bk_sb[:])
                            else:
                                nc.vector.tensor_copy(kT[:, toks], w_ps[:])
                        else:
                            v_st = vsp.tile([128, TW], f32, tag="vst")
                            if with_bias:
                                nc.vector.tensor_scalar_add(v_st[:], w_ps[:], bv_sb[:])
                            else:
                                nc.vector.tensor_copy(v_st[:], w_ps[:])
                            # PE transpose into a psum tile sharing the "pj"
                            # tag (no extra bank), then scatter into V_aug
                            vt_ps = ps2.tile([128, TW], f32, tag="pj", name="vt", bufs=2)
                            for j4 in range(TW // 128):
                                nc.tensor.transpose(
                                    vt_ps[:, j4 * 128 : j4 * 128 + 128],
                                    v_st[:, j4 * 128 : j4 * 128 + 128],
                                    ident[:],
                                )
                            c = tg * 4 * VSTRIDE
                            dst = vaug[:, c : c + 4 * VSTRIDE].rearrange(
                                "p (t g w) -> p t g w", t=4, g=2
                            )[:, :, :, 0:HD]
                            srcv = vt_ps[:].rearrange("p (t g w) -> p t g w", t=4, g=2)
                            nc.vector.tensor_copy(dst, srcv)

                def emit_outproj_tg(tg):
                    # partial output projection for one 512-token group; emitted
                    # one batch late, interleaved into attention windows so its
                    # PE/copy work fills the exp/normalize dependency stalls.
                    tok = slice(tg * TW, tg * TW + TW)
                    for dmg in range(4):
                        pr = ps2.tile([128, 2 * TW], f32, tag="s", name="pr", bufs=2)
                        for dmi in range(2):
                            dm = dmg * 2 + dmi
                            s = slice(dm * 128, dm * 128 + 128)
                            nc.tensor.matmul(
                                pr[:, dmi * TW : dmi * TW + TW],
                                wo_sb[:, s],
                                oT[:, tok],
                                start=True,
                                stop=True,
                            )
                        st2 = outp.tile([128, 2 * TW], mybir.dt.float16, tag="st")
                        if dmg % 4 == 3:
                            nc.scalar.copy(st2[:], pr[:])
                        else:
                            nc.vector.tensor_copy(st2[:], pr[:])
                        for dmi in range(2):
                            dm = dmg * 2 + dmi
                            s = slice(dm * 128, dm * 128 + 128)
                            nc.sync.dma_start(
                                out[s, tok], st2[:, dmi * TW : dmi * TW + TW]
                            )

                for tg in range(4):
                    emit_proj_tg(tg)
                nc.sync.dma_start(wo_sb[:], wo[:, :])

                for b in range(B):
                    tb = b * T
                    for wi in range(NWIN):
                        win = slice(tb + wi * TW, tb + wi * TW + TW)
                        jmax = 4 * wi + 4
                        o_ps = [
                            ps2.tile([HD + 1, TW], f32, tag=f"o{h}", name=f"o{h}")
                            for h in (0, 1)
                        ]
                        # j's processed in software-pipelined pairs: scores of
                        # pair c (64-row mode back-to-back), exps of c, then PV
                        # accumulations of pair c-1 (128-row mode) — halves the
                        # PE row-tiling mode switches and keeps the PE off the
                        # exp critical path.
                        def emit_pvs(plist):
                            for j, c0, p_pr in plist:
                                vcol = ((tb // 128) + j) * VSTRIDE
                                for h in (0, 1):
                                    nc.tensor.matmul(
                                        o_ps[h][:, c0:TW],
                                        vaug[:, vcol + h * (HD + 1) : vcol + h * (HD + 1) + HD + 1],
                                        p_pr[:, h * TW + c0 : h * TW + TW],
                                        start=(j == 0),
                                        stop=(j == jmax - 1),
                                    )

                        prev = []
                        for jc in range(0, jmax, 2):
                            cur = []
                            for j in range(jc, min(jc + 2, jmax)):
                                bj = slice(tb + j * 128, tb + j * 128 + 128)
                                d = j - 4 * wi
                                c0 = 128 * d if d > 0 else 0
                                s_pr = ps2.tile([128, 2 * TW], f32, tag="s", bufs=2)
                                for h in (0, 1):
                                    nc.tensor.matmul(
                                        s_pr[:, h * TW + c0 : h * TW + TW],
                                        kT[h * HD : h * HD + HD, bj],
                                        qT[h * HD : h * HD + HD, win.start + c0 : win.stop],
                                        start=True,
                                        stop=True,
                                    )
                                cur.append((j, d, c0, s_pr))
                            pcur = []
                            for j, d, c0, s_pr in cur:
                                p_pr = ppool.tile([128, 2 * TW], bf16, tag="p", bufs=6)
                                if c0 == 0:
                                    nc.scalar.activation(
                                        p_pr[:],
                                        s_pr[:],
                                        mybir.ActivationFunctionType.Exp,
                                        scale=float(SCALE),
                                    )
                                else:
                                    p_v = p_pr[:].rearrange("p (g w) -> p g w", g=2)[:, :, c0:TW]
                                    s_v = s_pr[:].rearrange("p (g w) -> p g w", g=2)[:, :, c0:TW]
                                    nc.scalar.activation(
                                        p_v,
                                        s_v,
                                        mybir.ActivationFunctionType.Exp,
                                        scale=float(SCALE),
                                    )
                                if d >= 0:  # diagonal: zero strict lower triangle
                                    for h in (0, 1):
                                        ts = slice(
                                            h * TW + 128 * d, h * TW + 128 * d + 128
                                        )
                                        nc.vector.tensor_tensor(
                                            p_pr[:, ts], p_pr[:, ts], tri_sb[:], MULT
                                        )
                                pcur.append((j, c0, p_pr))
                            emit_pvs(prev)
                            prev = pcur
                        emit_pvs(prev)
                        for h in (0, 1):
                            o_st = nrm.tile([HD + 1, TW], f32, tag="ost", bufs=6)
                            nc.vector.tensor_copy(o_st[:], o_ps[h][:])
                            den0 = nrm.tile([1, TW], f32, tag="den0", bufs=3)
                            nc.sync.dma_start(den0[:], o_st[HD : HD + 1, :])
                            bc = nrm.tile([HD, TW], f32, tag="bc", bufs=3)
                            nc.gpsimd.partition_broadcast(bc[:], den0[0:1, :])
                            if debug and b == 0 and wi == 0 and h == 0:
                                nc.sync.dma_start(dbg_ost[:, :], o_st[:])
                            rc = nrm.tile([HD, TW], f32, tag="rc", bufs=3)
                            nc.vector.reciprocal_approx_fast(out=rc[:], in_=bc[:])
                            nc.vector.tensor_tensor(
                                oT[h * HD : h * HD + HD, win], o_st[0:HD, :], rc[:], MULT
                            )
                        if b < B - 1:
                            emit_proj_tg(4 * (b + 1) + wi)
                        if b > 0:
                            emit_outproj_tg(4 * (b - 1) + wi)

                for wi in range(NWIN):
                    emit_outproj_tg(4 * (B - 1) + wi)

            if debug:
                nc.sync.dma_start(dbg_qT[:, :], qT[:].bitcast(f32))
                nc.sync.dma_start(dbg_kT[:, :], kT[:].bitcast(f32))
                nc.sync.dma_start(dbg_va[:, :], vaug[:])
                nc.sync.dma_start(dbg_oT[:, :], oT[:])
                nc.sync.dma_start(dbg_tri[:, :], tri_sb[:])

    nc.compile()
    return nc


def _get_nc(with_bias: bool, debug: bool = False):
    key = (with_bias, debug)
    if key not in _cache:
        _cache[key] = _build(with_bias, debug)
    return _cache[key]


def _make_in_maps(x, Wq, bq, Wk, bk, Wv, bv, Wo, with_bias):
    bf = ml_dtypes.bfloat16
    xT = np.ascontiguousarray(x.reshape(BT, D).T.astype(bf))
    in_maps = []
    for c in range(NCORES):
        cs = slice(c * DPC, c * DPC + DPC)
        def _blockT(w):
            # [D, DPC] -> [128, D]: partition p holds, per kt block, row
            # (kt*128 + p) of w — the layout wq_sb[:, kt*128:+128] expects
            # (partitions = contraction slice, cols = output features).
            return np.ascontiguousarray(
                w.astype(bf).reshape(NKT, 128, DPC).transpose(1, 0, 2).reshape(128, D)
            )

        m = {
            "xT": xT,
            "wq": _blockT(Wq[:, cs]),
            "wk": _blockT(Wk[:, cs]),
            "wv": _blockT(Wv[:, cs]),
            "wo": np.ascontiguousarray(Wo[cs, :].astype(bf)),
        }
        if with_bias:
            m["bq"] = np.ascontiguousarray(bq[cs]).reshape(DPC, 1)
            m["bk"] = np.ascontiguousarray(bk[cs]).reshape(DPC, 1)
            m["bv"] = np.ascontiguousarray(bv[cs]).reshape(DPC, 1)
        in_maps.append(m)
    return in_maps


def _gather(res, bo):
    acc = np.zeros((D, BT), dtype=np.float32)
    for r in res.results:
        acc += r["out"].astype(np.float32)
    y = acc.T + bo[None, :]
    return np.ascontiguousarray(y.reshape(B, T, D), dtype=np.float32)


def kernel(x, Wq, bq, Wk, bk, Wv, bv, Wo, bo, _trace=False):
    x = np.asarray(x, dtype=np.float32)
    Wq, Wk, Wv, Wo = (np.asarray(w, dtype=np.float32) for w in (Wq, Wk, Wv, Wo))
    bq, bk, bv, bo = (np.asarray(b_, dtype=np.float32) for b_ in (bq, bk, bv, bo))

    with_bias = bool(np.any(bq != 0) or np.any(bk != 0) or np.any(bv != 0))
    nc = _get_nc(with_bias)
    in_maps = _make_in_maps(x, Wq, bq, Wk, bk, Wv, bv, Wo, with_bias)
    res = run_bass_kernel_spmd(
        nc, in_maps, core_ids=list(range(NCORES)), trace=_trace
    )
    y = _gather(res, bo)
    if _trace:
        return y, res
    return y
